# revision 1
# baseline (speedup 1.0000x reference)
"""Bahdanau-style additive attention on 8 TRN2 NeuronCores.

  hidden = tanh(q @ Wq + k @ Wk)        (B, L, H)
  scores = hidden @ v_param             (B, L)
  attn   = softmax(scores, axis=-1)
  out    = attn @ v                     (B, D)

Sharding: data-parallel over batch — 4 batches per core (B=32, 8 cores).

Per-core device pipeline:

  W1  preT[H, L]   = Wk.T @ kT          stationary=Wk, moving=host-transposed k
  ACT hiddenT      = tanh(preT + qWq_b) per-partition bias
  W2  scores[L, 1] = hiddenT.T @ vp     stationary=hidden chunk -> score COLUMNS
  ACT w = exp(scores)                   no max-subtraction (|scores| << 88)
  W3  acc[1, D+1]  = w.T @ [v | 1]      stationary=w column (float32r), the
                                        ones column gives the softmax
                                        denominator for free
  host: out = acc[:D] / acc[D]

MODE="hilo": W1/W2 run as bf16 hi+lo split pairs (x = hi + lo exactly to
~2^-17), 3 matmuls each with the negligible lo*lo term dropped — fp32-grade
precision at bf16 PE speed, same DMA bytes as fp32.
MODE="f32r": W1/W2 in float32r (TF32-like, ~11-bit-mantissa RNE) — fewer
PE instructions, ~1e-3 relative error.
"""

import ml_dtypes
import numpy as np

import concourse.bass as bass
import concourse.mybir as mybir
from concourse.tile import TileContext

B, L, D, H = 32, 8192, 128, 128
NCORES = 8
BPC = B // NCORES  # batches per core
CHUNK = 512  # L positions per W1/tanh chunk (psum bank limit)
NCH = L // CHUNK  # 16 chunks per batch
KTILE = 2048  # L positions per kT DMA tile
KCH = KTILE // CHUNK  # W1 chunks per kT tile
SUB = 128  # L positions per W2/W3 sub-chunk (stationary width)
NSUB = CHUNK // SUB  # 4
DV = 132  # v row: 128 data + ones col + 3 pad
VT_COLS = 16  # W3 sub-chunks per v SBUF tile
NVT = L // (SUB * VT_COLS)  # 4 v tiles per batch

MODE = "hilo"  # "hilo" | "f32r"

F32 = mybir.dt.float32
F32R = mybir.dt.float32r
BF16 = mybir.dt.bfloat16
ACTF = mybir.ActivationFunctionType
ALU = mybir.AluOpType

_CACHE = {}


def _split_excess_waits(nc, max_waits=1):
    """walrus in this env accepts at most one sync-wait per instruction;
    move extras onto InstNoOps placed just before (same engine, in order)."""
    for fn in nc.m.functions:
        for bb in fn.blocks:
            insts = list(bb.instructions)
            new_insts = []
            for ins in insts:
                si = ins.sync_info
                waits = list(si.on_wait) if si and si.on_wait else []
                if len(waits) > max_waits:
                    extra, keep = waits[:-max_waits], waits[-max_waits:]
                    for g0 in range(0, len(extra), max_waits):
                        pre = mybir.InstNoOp(
                            name=f"{ins.name}-waitsplit{g0}",
                            engine=ins.engine,
                            ins=[],
                            outs=[],
                            sync_info=mybir.SyncInfo(
                                on_wait=extra[g0 : g0 + max_waits], on_update=[]
                            ),
                        )
                        nc.register_instruction(pre, overwrite=True)
                        new_insts.append(pre)
                    ins.sync_info = mybir.SyncInfo(
                        on_wait=keep, on_update=list(si.on_update or [])
                    )
                new_insts.append(ins)
            if len(new_insts) != len(insts):
                bb.instructions[:] = new_insts


def build_nc(mode=MODE):
    nc = bass.Bass("TRN2")
    hilo = mode == "hilo"

    if hilo:
        kh_in = nc.dram_tensor("kh", [BPC, D, L], BF16, kind="ExternalInput")
        kl_in = nc.dram_tensor("kl", [BPC, D, L], BF16, kind="ExternalInput")
        # packed consts: cols 0:4 qwq (f32), 4:68 wkh, 68:132 wkl (bf16 pairs),
        # 132 vph|vpl interleaved as one f32 col
        cst_in = nc.dram_tensor("cst", [128, 133], F32, kind="ExternalInput")
    else:
        kT_in = nc.dram_tensor("kT", [BPC, D, L], F32R, kind="ExternalInput")
        wk_in = nc.dram_tensor("wk", [D, H], F32R, kind="ExternalInput")
        vp_in = nc.dram_tensor("vp", [H, 4], F32R, kind="ExternalInput")
    v_in = nc.dram_tensor("vv", [BPC, NVT, SUB, VT_COLS * DV], F32R, kind="ExternalInput")
    if not hilo:
        qwq_in = nc.dram_tensor("qwq", [H, BPC], F32, kind="ExternalInput")
    out_d = nc.dram_tensor("out", [1, BPC * DV], F32, kind="ExternalOutput")

    with TileContext(nc) as tc:
        with (
            tc.tile_pool(name="const", bufs=1) as cpool,
            tc.tile_pool(name="kp", bufs=8) as kpool,
            tc.tile_pool(name="vp_", bufs=2 * NVT) as vpool,
            tc.tile_pool(name="hp", bufs=4) as hpool,
            tc.tile_pool(name="wp", bufs=2) as wpool,
            tc.tile_pool(name="ob", bufs=1) as opool,
            tc.tile_pool(name="pre", bufs=2, space="PSUM") as pre_pool,
            tc.tile_pool(name="sps", bufs=2, space="PSUM") as s_pool,
            tc.tile_pool(name="ops", bufs=2, space="PSUM") as o_pool,
        ):
            # HAM warm-up on zeroed tiles: needs no DMA, so the PE clock
            # gate lifts during the Tile preamble / first transfers.
            zwarm = cpool.tile([128, 512], BF16)
            nc.gpsimd.memset(zwarm[:], 0.0)
            warm_ps = pre_pool.tile([H, CHUNK], F32, tag="pre")
            for _ in range(16):
                nc.tensor.matmul(
                    warm_ps[:, :512], zwarm[:, :128], zwarm[:], start=True, stop=True
                )

            if hilo:
                cst = cpool.tile([128, 133], F32)
                nc.sync.dma_start(cst[:], cst_in[:])
                qwq = cst[:, 0:4]
                wkh = cst[:, 4:68].bitcast(BF16)
                wkl = cst[:, 68:132].bitcast(BF16)
                vph = cst[:, 132:133].bitcast(BF16)[:, 0:1]
                vpl = cst[:, 132:133].bitcast(BF16)[:, 1:2]
            else:
                qwq = cpool.tile([H, BPC], F32)
                nc.sync.dma_start(qwq[:], qwq_in[:])
                wk = cpool.tile([D, H], F32R)
                vp4 = cpool.tile([H, 4], F32R)
                nc.sync.dma_start(wk[:], wk_in[:])
                nc.sync.dma_start(vp4[:], vp_in[:])

            out_sb = opool.tile([1, BPC * DV], F32)

            def load_ktile(b, t):
                if hilo:
                    kht = kpool.tile([D, KTILE], BF16, tag="kht")
                    klt = kpool.tile([D, KTILE], BF16, tag="klt")
                    nc.sync.dma_start(kht[:], kh_in[b, :, t * CHUNK : t * CHUNK + KTILE])
                    nc.sync.dma_start(klt[:], kl_in[b, :, t * CHUNK : t * CHUNK + KTILE])
                    return (kht, klt)
                ktile = kpool.tile([D, KTILE], F32R, tag="kt")
                nc.sync.dma_start(ktile[:], kT_in[b, :, t * CHUNK : t * CHUNK + KTILE])
                return ktile

            for b in range(BPC):
                # v~ tiles for this batch (SWDGE queue so the large v
                # prefetches never head-of-line-block the kT stream, whose
                # issue rate is throttled by PE back-pressure)
                kts = {}
                if b == 0:
                    kts[0] = load_ktile(0, 0)
                    v_tiles = [None] * NVT
                else:
                    v_tiles = next_v_tiles
                next_v_tiles = [None] * NVT

                acc = o_pool.tile([1, DV], F32, tag="acc")
                w = wpool.tile([SUB, L // SUB], F32R, tag="w")
                for t in range(NCH):
                    if t % KCH == 0 and t // KCH not in kts:
                        kts[t // KCH] = load_ktile(b, t)
                    # batch 0 pulls its own v spread through its front half
                    if b == 0 and t % 2 == 0 and t // 2 < NVT:
                        vtile = vpool.tile([SUB, VT_COLS * DV], F32R, tag="vt")
                        nc.gpsimd.dma_start(vtile[:], v_in[0, t // 2])
                        v_tiles[t // 2] = vtile
                    # prefetch next batch's v in the BACK half of this batch,
                    # when the k lookahead buffers are already full
                    voff = NCH - 2 * NVT
                    if t >= voff and (t - voff) % 2 == 0 and b + 1 < BPC:
                        vt = (t - voff) // 2
                        vtile = vpool.tile([SUB, VT_COLS * DV], F32R, tag="vt")
                        nc.gpsimd.dma_start(vtile[:], v_in[b + 1, vt])
                        next_v_tiles[vt] = vtile

                    cs = slice((t % KCH) * CHUNK, (t % KCH + 1) * CHUNK)
                    pre = pre_pool.tile([H, CHUNK], F32, tag="pre")
                    if hilo:
                        kht, klt = kts[t // KCH]
                        nc.tensor.matmul(
                            pre[:], wkh[:], kht[:, cs], start=True, stop=False
                        )
                        nc.tensor.matmul(
                            pre[:], wkl[:], kht[:, cs], start=False, stop=False
                        )
                        nc.tensor.matmul(
                            pre[:], wkh[:], klt[:, cs], start=False, stop=True
                        )
                    else:
                        nc.tensor.matmul(
                            pre[:], wk[:], kts[t // KCH][:, cs], start=True, stop=True
                        )

                    if hilo:
                        h32 = hpool.tile([H, CHUNK], F32, tag="h32")
                        nc.scalar.activation(
                            h32[:], pre[:], ACTF.Tanh, bias=qwq[:, b : b + 1],
                            scale=1.0,
                        )
                        hh = hpool.tile([H, CHUNK], BF16, tag="hh")
                        nc.vector.tensor_copy(hh[:], h32[:])
                        hl = hpool.tile([H, CHUNK], BF16, tag="hl")
                        nc.vector.tensor_sub(hl[:], h32[:], hh[:])
                        if t % 4 == 0:
                            scol4 = s_pool.tile([SUB, 4 * NSUB], F32, tag="scol")
                        scol = scol4[:, (t % 4) * NSUB : (t % 4 + 1) * NSUB]
                        for j in range(NSUB):
                            js = slice(j * SUB, (j + 1) * SUB)
                            nc.tensor.matmul(
                                scol[:, j : j + 1], hh[:, js], vph[:],
                                start=True, stop=False,
                            )
                            nc.tensor.matmul(
                                scol[:, j : j + 1], hh[:, js], vpl[:],
                                start=False, stop=False,
                            )
                            nc.tensor.matmul(
                                scol[:, j : j + 1], hl[:, js], vph[:],
                                start=False, stop=True,
                            )
                        if t % 4 == 3:
                            nc.scalar.activation(
                                w[:, NSUB * (t - 3) : NSUB * (t + 1)],
                                scol4[:],
                                ACTF.Exp,
                            )
                    else:
                        hid = hpool.tile([H, CHUNK], F32R, tag="hid")
                        nc.scalar.activation(
                            hid[:], pre[:], ACTF.Tanh, bias=qwq[:, b : b + 1],
                            scale=1.0,
                        )
                        scol = s_pool.tile([SUB, 4 * NSUB], F32, tag="scol")
                        for j in range(NSUB):
                            nc.tensor.matmul(
                                scol[:, 4 * j : 4 * j + 4],
                                hid[:, j * SUB : (j + 1) * SUB],
                                vp4[:],
                                start=True,
                                stop=True,
                            )
                        nc.scalar.activation(
                            w[:, NSUB * t : NSUB * (t + 1)],
                            scol[:, 0 : 4 * NSUB : 4],
                            ACTF.Exp,
                        )

                nsub_total = L // SUB
                for tp in range(nsub_total):
                    vt, col = divmod(tp, VT_COLS)
                    nc.tensor.matmul(
                        acc[:],
                        w[:, tp : tp + 1],
                        v_tiles[vt][:, col * DV : (col + 1) * DV],
                        start=(tp == 0),
                        stop=(tp == nsub_total - 1),
                    )
                nc.scalar.copy(out_sb[:, b * DV : (b + 1) * DV], acc[:])

            nc.sync.dma_start(out_d[:], out_sb[:])

    _split_excess_waits(nc)
    return nc


def _prep_inputs(q, k, v, W_line, v_param, mode=MODE):
    """Host-side shard + layout prep. Returns per-core input maps."""
    hilo = mode == "hilo"
    qWq = q.astype(np.float64) @ W_line[:D].astype(np.float64)  # (B, H)
    wk = np.ascontiguousarray(W_line[D:]).astype(np.float32)  # (D, H)

    if hilo:
        wkh = np.ascontiguousarray(wk.astype(ml_dtypes.bfloat16))
        wkl = np.ascontiguousarray(
            (wk - wkh.astype(np.float32)).astype(ml_dtypes.bfloat16)
        )
        vph = v_param.astype(ml_dtypes.bfloat16)
        vpl = (v_param - vph.astype(np.float32)).astype(ml_dtypes.bfloat16)
        vpair = np.ascontiguousarray(
            np.stack([vph, vpl], axis=1)
        )  # [H, 2] bf16 -> one f32 col
    else:
        vp4 = np.tile(v_param[:, None], (1, 4)).astype(np.float32)

    in_maps = []
    for c in range(NCORES):
        bs = slice(c * BPC, (c + 1) * BPC)
        kT = np.ascontiguousarray(k[bs].transpose(0, 2, 1))  # (BPC, D, L)
        vv = np.zeros((BPC, L, DV), dtype=np.float32)
        vv[:, :, :D] = v[bs]
        vv[:, :, D] = 1.0
        # permute into the SBUF tile layout: [b][vt][p][t*DV+d]
        vv = np.ascontiguousarray(
            vv.reshape(BPC, NVT, VT_COLS, SUB, DV)
            .transpose(0, 1, 3, 2, 4)
            .reshape(BPC, NVT, SUB, VT_COLS * DV)
        )
        qwq = np.ascontiguousarray(qWq[bs].T.astype(np.float32))  # (H, BPC)
        m = {"vv": vv}
        if hilo:
            kh = kT.astype(ml_dtypes.bfloat16)
            kl = (kT - kh.astype(np.float32)).astype(ml_dtypes.bfloat16)
            cst = np.zeros((128, 133), dtype=np.float32)
            cst[:, 0:4] = qwq
            cst[:, 4:68] = wkh.view(np.float32)
            cst[:, 68:132] = wkl.view(np.float32)
            cst[:, 132:133] = vpair.view(np.float32)
            m.update(kh=kh, kl=kl, cst=cst)
        else:
            m.update(kT=kT, wk=wk, vp=vp4, qwq=qwq)
        in_maps.append(m)
    return in_maps


def _gather_output(results):
    out = np.empty((B, D), dtype=np.float32)
    for c, r in enumerate(results):
        rows = r["out"].reshape(BPC, DV).astype(np.float64)
        out[c * BPC : (c + 1) * BPC] = (rows[:, :D] / rows[:, D : D + 1]).astype(
            np.float32
        )
    return out


def run(q, k, v, W_line, v_param, trace=False, mode=MODE, **spmd_kwargs):
    from concourse.bass_utils import run_bass_kernel_spmd

    key = ("nc", mode)
    if key not in _CACHE:
        _CACHE[key] = build_nc(mode)
    nc = _CACHE[key]
    in_maps = _prep_inputs(q, k, v, W_line, v_param, mode)
    res = run_bass_kernel_spmd(
        nc, in_maps, list(range(NCORES)), trace=trace, **spmd_kwargs
    )
    return _gather_output(res.results), res


def kernel(q, k, v, W_line, v_param):
    out, _ = run(q, k, v, W_line, v_param, trace=False)
    return out



# revision 2
# speedup vs baseline: 1.7877x; 1.7877x over previous
"""Bahdanau-style additive attention on 8 TRN2 NeuronCores.

  hidden = tanh(q @ Wq + k @ Wk)        (B, L, H)
  scores = hidden @ v_param             (B, L)
  attn   = softmax(scores, axis=-1)
  out    = attn @ v                     (B, D)

Sharding: data-parallel over batch — 4 batches per core (B=32, 8 cores).

All-bf16 pipeline (k, Wk, hidden, w, v in bf16; f32 PSUM accumulate).
Measured end-to-end rel err ~8e-3 vs the fp32 reference (threshold 2e-2).
Halves HBM traffic vs fp32/hi-lo (16.3 MB/core) and keeps every matmul at
the 1-cycle/column bf16 PE rate.

Per-core device pipeline, per batch:

  W1  pre[H, C]   = Wk.T @ kT chunk     one bf16 matmul per 512-chunk
  ACT hh          = tanh(pre + qWq_b)   per-partition bias, bf16 out
  W2  scol[:, j]  = hh_js.T @ vph       4 one-col matmuls; hh stationary
                                        (128-col bf16 loads -> FWL 2x)
  ACT w = exp(scol4)                    per 4 chunks, [128, 16] -> bf16
  W3  acc[1, DV]  = w_col.T @ [v | 1]   64 matmuls, ones col gives the
                                        softmax denominator for free
  host: out = acc[:D] / acc[D]
"""

import ml_dtypes
import numpy as np

import concourse.bass as bass
import concourse.mybir as mybir
from concourse.tile import TileContext

B, L, D, H = 32, 8192, 128, 128
NCORES = 8
BPC = B // NCORES  # batches per core
CHUNK = 512  # L positions per W1/tanh chunk (psum bank limit)
NCH = L // CHUNK  # 16 chunks per batch
KTILE = 2048  # L positions per kT DMA tile
KCH = KTILE // CHUNK  # W1 chunks per kT tile
SUB = 128  # L positions per W2/W3 sub-chunk (stationary width)
NSUB = CHUNK // SUB  # 4
DV = 130  # v row: 128 data + ones col + 1 pad (4B align)
VT_COLS = 16  # W3 sub-chunks per v SBUF tile
NVT = L // (SUB * VT_COLS)  # 4 v tiles per batch

F32 = mybir.dt.float32
BF16 = mybir.dt.bfloat16
ACTF = mybir.ActivationFunctionType

_CACHE = {}


def _split_excess_waits(nc, max_waits=1):
    """walrus in this env accepts at most one sync-wait per instruction;
    move extras onto InstNoOps placed just before (same engine, in order)."""
    for fn in nc.m.functions:
        for bb in fn.blocks:
            insts = list(bb.instructions)
            new_insts = []
            for ins in insts:
                si = ins.sync_info
                waits = list(si.on_wait) if si and si.on_wait else []
                if len(waits) > max_waits:
                    extra, keep = waits[:-max_waits], waits[-max_waits:]
                    for g0 in range(0, len(extra), max_waits):
                        pre = mybir.InstNoOp(
                            name=f"{ins.name}-waitsplit{g0}",
                            engine=ins.engine,
                            ins=[],
                            outs=[],
                            sync_info=mybir.SyncInfo(
                                on_wait=extra[g0 : g0 + max_waits], on_update=[]
                            ),
                        )
                        nc.register_instruction(pre, overwrite=True)
                        new_insts.append(pre)
                    ins.sync_info = mybir.SyncInfo(
                        on_wait=keep, on_update=list(si.on_update or [])
                    )
                new_insts.append(ins)
            if len(new_insts) != len(insts):
                bb.instructions[:] = new_insts


def build_nc():
    nc = bass.Bass("TRN2")

    kh_in = nc.dram_tensor("kh", [BPC, D, L], BF16, kind="ExternalInput")
    # packed consts: cols 0:4 qwq (f32), 4:68 wk (bf16 pairs), 68 vph|0
    cst_in = nc.dram_tensor("cst", [128, 69], F32, kind="ExternalInput")
    v_in = nc.dram_tensor("vv", [BPC, NVT, SUB, VT_COLS * DV], BF16, kind="ExternalInput")
    out_d = nc.dram_tensor("out", [1, BPC * DV], F32, kind="ExternalOutput")

    with TileContext(nc) as tc:
        with (
            tc.tile_pool(name="const", bufs=1) as cpool,
            tc.tile_pool(name="kp", bufs=6) as kpool,
            tc.tile_pool(name="vp_", bufs=2 * NVT) as vpool,
            tc.tile_pool(name="hp", bufs=4) as hpool,
            tc.tile_pool(name="wp", bufs=2) as wpool,
            tc.tile_pool(name="ob", bufs=1) as opool,
            tc.tile_pool(name="pre", bufs=2, space="PSUM") as pre_pool,
            tc.tile_pool(name="sps", bufs=2, space="PSUM") as s_pool,
            tc.tile_pool(name="ops", bufs=2, space="PSUM") as o_pool,
        ):
            # HAM warm-up on zeroed tiles: needs no DMA, so the PE clock
            # gate lifts during the Tile preamble / first transfers.
            zwarm = cpool.tile([128, 512], BF16)
            nc.gpsimd.memset(zwarm[:], 0.0)
            warm_ps = pre_pool.tile([H, CHUNK], F32, tag="pre")
            for _ in range(16):
                nc.tensor.matmul(
                    warm_ps[:, :512], zwarm[:, :128], zwarm[:], start=True, stop=True
                )

            cst = cpool.tile([128, 69], F32)
            nc.sync.dma_start(cst[:], cst_in[:])
            qwq = cst[:, 0:4]
            wk = cst[:, 4:68].bitcast(BF16)
            vph = cst[:, 68:69].bitcast(BF16)[:, 0:1]

            out_sb = opool.tile([1, BPC * DV], F32)

            def load_ktile(b, t):
                ktile = kpool.tile([D, KTILE], BF16, tag="kt")
                nc.sync.dma_start(ktile[:], kh_in[b, :, t * CHUNK : t * CHUNK + KTILE])
                return ktile

            for b in range(BPC):
                # v~ tiles for this batch (SWDGE queue so the large v
                # prefetches never head-of-line-block the kT stream, whose
                # issue rate is throttled by PE back-pressure)
                kts = {}
                if b == 0:
                    kts[0] = load_ktile(0, 0)
                    v_tiles = [None] * NVT
                else:
                    v_tiles = next_v_tiles
                next_v_tiles = [None] * NVT

                acc = o_pool.tile([1, DV], F32, tag="acc")
                w = wpool.tile([SUB, L // SUB], BF16, tag="w")
                for t in range(NCH):
                    if t % KCH == 0 and t // KCH not in kts:
                        kts[t // KCH] = load_ktile(b, t)
                    # batch 0 pulls its own v spread through its front half
                    if b == 0 and t % 2 == 0 and t // 2 < NVT:
                        vtile = vpool.tile([SUB, VT_COLS * DV], BF16, tag="vt")
                        nc.gpsimd.dma_start(vtile[:], v_in[0, t // 2])
                        v_tiles[t // 2] = vtile
                    # prefetch next batch's v in the BACK half of this batch,
                    # when the k lookahead buffers are already full
                    voff = NCH - 2 * NVT
                    if t >= voff and (t - voff) % 2 == 0 and b + 1 < BPC:
                        vt = (t - voff) // 2
                        vtile = vpool.tile([SUB, VT_COLS * DV], BF16, tag="vt")
                        nc.gpsimd.dma_start(vtile[:], v_in[b + 1, vt])
                        next_v_tiles[vt] = vtile

                    cs = slice((t % KCH) * CHUNK, (t % KCH + 1) * CHUNK)
                    pre = pre_pool.tile([H, CHUNK], F32, tag="pre")
                    nc.tensor.matmul(
                        pre[:], wk[:], kts[t // KCH][:, cs], start=True, stop=True
                    )

                    hh = hpool.tile([H, CHUNK], BF16, tag="hh")
                    nc.scalar.activation(
                        hh[:], pre[:], ACTF.Tanh, bias=qwq[:, b : b + 1], scale=1.0
                    )
                    if t % 4 == 0:
                        scol4 = s_pool.tile([SUB, 4 * NSUB], F32, tag="scol")
                    scol = scol4[:, (t % 4) * NSUB : (t % 4 + 1) * NSUB]
                    for j in range(NSUB):
                        js = slice(j * SUB, (j + 1) * SUB)
                        nc.tensor.matmul(
                            scol[:, j : j + 1], hh[:, js], vph[:],
                            start=True, stop=True,
                        )
                    if t % 4 == 3:
                        nc.scalar.activation(
                            w[:, NSUB * (t - 3) : NSUB * (t + 1)], scol4[:], ACTF.Exp
                        )

                nsub_total = L // SUB
                for tp in range(nsub_total):
                    vt, col = divmod(tp, VT_COLS)
                    nc.tensor.matmul(
                        acc[:],
                        w[:, tp : tp + 1],
                        v_tiles[vt][:, col * DV : (col + 1) * DV],
                        start=(tp == 0),
                        stop=(tp == nsub_total - 1),
                    )
                nc.scalar.copy(out_sb[:, b * DV : (b + 1) * DV], acc[:])

            nc.sync.dma_start(out_d[:], out_sb[:])

    _split_excess_waits(nc)
    return nc


def _prep_inputs(q, k, v, W_line, v_param):
    """Host-side shard + layout prep. Returns per-core input maps."""
    bf = ml_dtypes.bfloat16
    qWq = q.astype(np.float64) @ W_line[:D].astype(np.float64)  # (B, H)
    wk = np.ascontiguousarray(W_line[D:]).astype(bf)  # (D, H) bf16
    vp_pad = np.zeros((H, 2), dtype=bf)
    vp_pad[:, 0] = v_param.astype(bf)

    cst_base = np.zeros((128, 69), dtype=np.float32)
    cst_base[:, 4:68] = wk.view(np.float32)
    cst_base[:, 68:69] = vp_pad.view(np.float32)

    in_maps = []
    for c in range(NCORES):
        bs = slice(c * BPC, (c + 1) * BPC)
        kh = np.ascontiguousarray(k[bs].transpose(0, 2, 1).astype(bf))  # (BPC, D, L)
        vv = np.zeros((BPC, L, DV), dtype=np.float32)
        vv[:, :, :D] = v[bs]
        vv[:, :, D] = 1.0
        # permute into the SBUF tile layout: [b][vt][p][t*DV+d]
        vv = np.ascontiguousarray(
            vv.reshape(BPC, NVT, VT_COLS, SUB, DV)
            .transpose(0, 1, 3, 2, 4)
            .reshape(BPC, NVT, SUB, VT_COLS * DV)
            .astype(bf)
        )
        cst = cst_base.copy()
        cst[:, 0:4] = qWq[bs].T.astype(np.float32)  # (H, BPC)
        in_maps.append({"kh": kh, "vv": vv, "cst": cst})
    return in_maps


def _gather_output(results):
    out = np.empty((B, D), dtype=np.float32)
    for c, r in enumerate(results):
        rows = r["out"].reshape(BPC, DV).astype(np.float64)
        out[c * BPC : (c + 1) * BPC] = (rows[:, :D] / rows[:, D : D + 1]).astype(
            np.float32
        )
    return out


def run(q, k, v, W_line, v_param, trace=False, **spmd_kwargs):
    from concourse.bass_utils import run_bass_kernel_spmd

    if "nc" not in _CACHE:
        _CACHE["nc"] = build_nc()
    nc = _CACHE["nc"]
    in_maps = _prep_inputs(q, k, v, W_line, v_param)
    res = run_bass_kernel_spmd(
        nc, in_maps, list(range(NCORES)), trace=trace, **spmd_kwargs
    )
    return _gather_output(res.results), res


def kernel(q, k, v, W_line, v_param):
    out, _ = run(q, k, v, W_line, v_param, trace=False)
    return out


# revision 4
# speedup vs baseline: 1.8114x; 1.0133x over previous
"""Bahdanau-style additive attention on 8 TRN2 NeuronCores.

  hidden = tanh(q @ Wq + k @ Wk)        (B, L, H)
  scores = hidden @ v_param             (B, L)
  attn   = softmax(scores, axis=-1)
  out    = attn @ v                     (B, D)

Sharding: data-parallel over batch — 4 batches per core (B=32, 8 cores).

All-bf16 pipeline (k, Wk, hidden, w, v in bf16; f32 PSUM accumulate).
Measured end-to-end rel err ~9e-3 vs the fp32 reference (threshold 2e-2).

Per-core device pipeline, per batch (SC = 1024-position super-chunk):

  W1  pre[H, SC]  = Wk.T @ kT           two bf16 matmuls per super-chunk
  ACT hh          = tanh(pre + qWq_b)   one 1024-wide activation, bf16 out
  W2  scol[:, j]  = hh_js.T @ vph       8 one-col matmuls; hh stationary
  ACT w = exp(scol)                     per 2 super-chunks, [128,16] -> bf16
  W3  acc[32c, :] = w_col.T @ [v | 1]   4 col-group-packed accumulation
                                        chains (PSUM partitions 0/32/64/96)
                                        interleaved right after each exp;
                                        ones col gives the denominator
  DVE copies acc -> out rows; host sums the 4 partials and divides.
"""

import ml_dtypes
import numpy as np

import concourse.bass as bass
import concourse.mybir as mybir
from concourse.tile import TileContext

B, L, D, H = 32, 8192, 128, 128
NCORES = 8
BPC = B // NCORES  # batches per core
SC = 1024  # L positions per W1/tanh super-chunk (2 psum banks)
NSC = L // SC  # 8 super-chunks per batch
KTILE = 2048  # L positions per kT DMA tile
KSC = KTILE // SC  # super-chunks per kT tile
SUB = 128  # L positions per W2/W3 sub-chunk (stationary width)
NSUB = SC // SUB  # 8
DV = 130  # v row: 128 data + ones col + 1 pad (4B align)
VT_COLS = 16  # W3 sub-chunks per v SBUF tile
NVT = L // (SUB * VT_COLS)  # 4 v tiles per batch
NCHAIN = 4  # W3 col-group accumulation chains

F32 = mybir.dt.float32
BF16 = mybir.dt.bfloat16
ACTF = mybir.ActivationFunctionType

_CACHE = {}


def _split_excess_waits(nc, max_waits=1):
    """walrus in this env accepts at most one sync-wait per instruction;
    move extras onto InstNoOps placed just before (same engine, in order)."""
    for fn in nc.m.functions:
        for bb in fn.blocks:
            insts = list(bb.instructions)
            new_insts = []
            for ins in insts:
                si = ins.sync_info
                waits = list(si.on_wait) if si and si.on_wait else []
                if len(waits) > max_waits:
                    extra, keep = waits[:-max_waits], waits[-max_waits:]
                    for g0 in range(0, len(extra), max_waits):
                        pre = mybir.InstNoOp(
                            name=f"{ins.name}-waitsplit{g0}",
                            engine=ins.engine,
                            ins=[],
                            outs=[],
                            sync_info=mybir.SyncInfo(
                                on_wait=extra[g0 : g0 + max_waits], on_update=[]
                            ),
                        )
                        nc.register_instruction(pre, overwrite=True)
                        new_insts.append(pre)
                    ins.sync_info = mybir.SyncInfo(
                        on_wait=keep, on_update=list(si.on_update or [])
                    )
                new_insts.append(ins)
            if len(new_insts) != len(insts):
                bb.instructions[:] = new_insts


def build_nc():
    nc = bass.Bass("TRN2")

    kh_in = nc.dram_tensor("kh", [BPC, D, L], BF16, kind="ExternalInput")
    # packed consts: cols 0:4 qwq (f32), 4:68 wk (bf16 pairs), 68 vph|0
    cst_in = nc.dram_tensor("cst", [128, 69], F32, kind="ExternalInput")
    v_in = nc.dram_tensor("vv", [BPC, NVT, SUB, VT_COLS * DV], BF16, kind="ExternalInput")
    out_d = nc.dram_tensor("out", [128, BPC * DV], F32, kind="ExternalOutput")

    with TileContext(nc) as tc:
        with (
            tc.tile_pool(name="const", bufs=1) as cpool,
            tc.tile_pool(name="kp", bufs=6) as kpool,
            tc.tile_pool(name="vp_", bufs=2 * NVT) as vpool,
            tc.tile_pool(name="hp", bufs=3) as hpool,
            tc.tile_pool(name="wp", bufs=2) as wpool,
            tc.tile_pool(name="ob", bufs=1) as opool,
            tc.tile_pool(name="pre", bufs=2, space="PSUM") as pre_pool,
            tc.tile_pool(name="sps", bufs=2, space="PSUM") as s_pool,
            tc.tile_pool(name="ops", bufs=2, space="PSUM") as o_pool,
        ):
            # HAM warm-up on zeroed tiles: needs no DMA, so the PE clock
            # gate lifts during the Tile preamble / first transfers.
            zwarm = cpool.tile([128, 512], BF16)
            nc.gpsimd.memset(zwarm[:], 0.0)
            warm_ps = pre_pool.tile([H, SC], F32, tag="pre")
            for _ in range(12):
                nc.tensor.matmul(
                    warm_ps[:, :512], zwarm[:, :128], zwarm[:], start=True, stop=True
                )

            cst = cpool.tile([128, 69], F32)
            nc.sync.dma_start(cst[:], cst_in[:])
            qwq = cst[:, 0:4]
            wk = cst[:, 4:68].bitcast(BF16)
            vph = cst[:, 68:69].bitcast(BF16)[:, 0:1]

            out_sb = opool.tile([128, BPC * DV], F32)

            def load_ktile(b, g):
                ktile = kpool.tile([D, KTILE], BF16, tag="kt")
                nc.sync.dma_start(ktile[:], kh_in[b, :, g * KTILE : (g + 1) * KTILE])
                return ktile

            for b in range(BPC):
                # v~ tiles for this batch (SWDGE queue so the large v
                # prefetches never head-of-line-block the kT stream, whose
                # issue rate is throttled by PE back-pressure)
                kts = {}
                if b == 0:
                    kts[0] = load_ktile(0, 0)
                    v_tiles = [None] * NVT
                else:
                    v_tiles = next_v_tiles
                next_v_tiles = [None] * NVT

                acc = o_pool.tile([128, DV], F32, tag="acc")
                nc.vector.memset(acc[:], 0.0)
                w = wpool.tile([SUB, L // SUB], BF16, tag="w")
                for t in range(NSC):
                    if t % KSC == 0 and t // KSC not in kts:
                        kts[t // KSC] = load_ktile(b, t // KSC)
                    # batch 0 pulls its own v spread through its front half
                    if b == 0 and t < NVT:
                        vtile = vpool.tile([SUB, VT_COLS * DV], BF16, tag="vt")
                        nc.gpsimd.dma_start(vtile[:], v_in[0, t])
                        v_tiles[t] = vtile
                    # prefetch next batch's v in the back half of this batch
                    if t >= NSC - NVT and b + 1 < BPC:
                        vt = t - (NSC - NVT)
                        vtile = vpool.tile([SUB, VT_COLS * DV], BF16, tag="vt")
                        nc.gpsimd.dma_start(vtile[:], v_in[b + 1, vt])
                        next_v_tiles[vt] = vtile

                    pre = pre_pool.tile([H, SC], F32, tag="pre")
                    kt = kts[t // KSC]
                    off = (t % KSC) * SC
                    nc.tensor.matmul(
                        pre[:, 0:512], wk[:], kt[:, off : off + 512],
                        start=True, stop=True,
                    )
                    nc.tensor.matmul(
                        pre[:, 512:1024], wk[:], kt[:, off + 512 : off + 1024],
                        start=True, stop=True,
                    )

                    hh = hpool.tile([H, SC], BF16, tag="hh")
                    nc.scalar.activation(
                        hh[:], pre[:], ACTF.Tanh, bias=qwq[:, b : b + 1], scale=1.0
                    )
                    if t % 2 == 0:
                        scol2 = s_pool.tile([SUB, 2 * NSUB], F32, tag="scol")
                    scol = scol2[:, (t % 2) * NSUB : (t % 2 + 1) * NSUB]
                    for j in range(NSUB):
                        js = slice(j * SUB, (j + 1) * SUB)
                        nc.tensor.matmul(
                            scol[:, j : j + 1], hh[:, js], vph[:],
                            start=True, stop=True,
                        )
                    if t % 2 == 1:
                        g = t // 2  # w col group 16g..16g+15 == v tile g
                        nc.scalar.activation(
                            w[:, 16 * g : 16 * (g + 1)], scol2[:], ACTF.Exp
                        )
                        # W3 for this group, 4 col-group-packed chains
                        for col in range(VT_COLS):
                            tp = 16 * g + col
                            c = tp % NCHAIN
                            nc.tensor.matmul(
                                acc[32 * c : 32 * c + 1, :],
                                w[:, tp : tp + 1],
                                v_tiles[g][:, col * DV : (col + 1) * DV],
                                start=(tp < NCHAIN),
                                stop=(tp >= L // SUB - NCHAIN),
                                tile_position=(0, 32 * c),
                            )
                nc.vector.tensor_copy(out_sb[:, b * DV : (b + 1) * DV], acc[:])

            nc.sync.dma_start(out_d[:], out_sb[:])

    _split_excess_waits(nc)
    return nc


def _prep_inputs(q, k, v, W_line, v_param):
    """Host-side shard + layout prep. Returns per-core input maps."""
    bf = ml_dtypes.bfloat16
    qWq = q.astype(np.float64) @ W_line[:D].astype(np.float64)  # (B, H)
    wk = np.ascontiguousarray(W_line[D:]).astype(bf)  # (D, H) bf16
    vp_pad = np.zeros((H, 2), dtype=bf)
    vp_pad[:, 0] = v_param.astype(bf)

    cst_base = np.zeros((128, 69), dtype=np.float32)
    cst_base[:, 4:68] = wk.view(np.float32)
    cst_base[:, 68:69] = vp_pad.view(np.float32)

    in_maps = []
    for c in range(NCORES):
        bs = slice(c * BPC, (c + 1) * BPC)
        kh = np.ascontiguousarray(k[bs].transpose(0, 2, 1).astype(bf))  # (BPC, D, L)
        vv = np.zeros((BPC, L, DV), dtype=np.float32)
        vv[:, :, :D] = v[bs]
        vv[:, :, D] = 1.0
        # permute into the SBUF tile layout: [b][vt][p][t*DV+d]
        vv = np.ascontiguousarray(
            vv.reshape(BPC, NVT, VT_COLS, SUB, DV)
            .transpose(0, 1, 3, 2, 4)
            .reshape(BPC, NVT, SUB, VT_COLS * DV)
            .astype(bf)
        )
        cst = cst_base.copy()
        cst[:, 0:4] = qWq[bs].T.astype(np.float32)  # (H, BPC)
        in_maps.append({"kh": kh, "vv": vv, "cst": cst})
    return in_maps


def _gather_output(results):
    out = np.empty((B, D), dtype=np.float32)
    for c, r in enumerate(results):
        rows = r["out"].reshape(128, BPC, DV).astype(np.float64)
        # sum the 4 col-group partial accumulators (partitions 0/32/64/96)
        s = rows[0] + rows[32] + rows[64] + rows[96]  # (BPC, DV)
        out[c * BPC : (c + 1) * BPC] = (s[:, :D] / s[:, D : D + 1]).astype(np.float32)
    return out


def run(q, k, v, W_line, v_param, trace=False, **spmd_kwargs):
    from concourse.bass_utils import run_bass_kernel_spmd

    if "nc" not in _CACHE:
        _CACHE["nc"] = build_nc()
    nc = _CACHE["nc"]
    in_maps = _prep_inputs(q, k, v, W_line, v_param)
    res = run_bass_kernel_spmd(
        nc, in_maps, list(range(NCORES)), trace=trace, **spmd_kwargs
    )
    return _gather_output(res.results), res


def kernel(q, k, v, W_line, v_param):
    out, _ = run(q, k, v, W_line, v_param, trace=False)
    return out


# revision 5
# speedup vs baseline: 1.8703x; 1.0325x over previous
"""Bahdanau-style additive attention on 8 TRN2 NeuronCores.

  hidden = tanh(q @ Wq + k @ Wk)        (B, L, H)
  scores = hidden @ v_param             (B, L)
  attn   = softmax(scores, axis=-1)
  out    = attn @ v                     (B, D)

Sharding: data-parallel over batch — 4 batches per core (B=32, 8 cores).

All-bf16 pipeline (k, Wk, hidden, w, v in bf16; f32 PSUM accumulate).
Measured end-to-end rel err ~9e-3 vs the fp32 reference (threshold 2e-2).

Per-core device pipeline, per batch (SC = 1024-position super-chunk):

  W1  pre[H, SC]  = Wk.T @ kT           two bf16 matmuls per super-chunk
  ACT hh          = tanh(pre + qWq_b)   one 1024-wide activation, bf16 out
  W2  scol[:, j]  = hh_js.T @ vph       8 one-col matmuls; hh stationary
  ACT w = exp(scol)                     per 2 super-chunks, [128,16] -> bf16
  W3  acc[32c, :] = w_col.T @ [v | 1]   4 col-group-packed accumulation
                                        chains (PSUM partitions 0/32/64/96)
                                        interleaved right after each exp;
                                        ones col gives the denominator
  DVE copies acc -> out rows; host sums the 4 partials and divides.
"""

import ml_dtypes
import numpy as np

import concourse.bass as bass
import concourse.mybir as mybir
from concourse.tile import TileContext

B, L, D, H = 32, 8192, 128, 128
NCORES = 8
BPC = B // NCORES  # batches per core
SC = 512  # L positions per W1/tanh chunk (1 psum bank)
NSC = L // SC  # 16 chunks per batch
KTILE = 2048  # L positions per kT DMA tile
KSC = KTILE // SC  # super-chunks per kT tile
SUB = 128  # L positions per W2/W3 sub-chunk (stationary width)
NSUB = SC // SUB  # 4
DV = 130  # v row: 128 data + ones col + 1 pad (4B align)
VT_COLS = 16  # W3 sub-chunks per v SBUF tile
NVT = L // (SUB * VT_COLS)  # 4 v tiles per batch
NCHAIN = 4  # W3 col-group accumulation chains

F32 = mybir.dt.float32
BF16 = mybir.dt.bfloat16
ACTF = mybir.ActivationFunctionType

_CACHE = {}


def _split_excess_waits(nc, max_waits=1):
    """walrus in this env accepts at most one sync-wait per instruction;
    move extras onto InstNoOps placed just before (same engine, in order)."""
    for fn in nc.m.functions:
        for bb in fn.blocks:
            insts = list(bb.instructions)
            new_insts = []
            for ins in insts:
                si = ins.sync_info
                waits = list(si.on_wait) if si and si.on_wait else []
                if len(waits) > max_waits:
                    extra, keep = waits[:-max_waits], waits[-max_waits:]
                    for g0 in range(0, len(extra), max_waits):
                        pre = mybir.InstNoOp(
                            name=f"{ins.name}-waitsplit{g0}",
                            engine=ins.engine,
                            ins=[],
                            outs=[],
                            sync_info=mybir.SyncInfo(
                                on_wait=extra[g0 : g0 + max_waits], on_update=[]
                            ),
                        )
                        nc.register_instruction(pre, overwrite=True)
                        new_insts.append(pre)
                    ins.sync_info = mybir.SyncInfo(
                        on_wait=keep, on_update=list(si.on_update or [])
                    )
                new_insts.append(ins)
            if len(new_insts) != len(insts):
                bb.instructions[:] = new_insts


def build_nc():
    nc = bass.Bass("TRN2")

    kh_in = nc.dram_tensor("kh", [BPC, D, L], BF16, kind="ExternalInput")
    # packed consts: cols 0:4 qwq (f32), 4:68 wk (bf16 pairs), 68 vph|0
    cst_in = nc.dram_tensor("cst", [128, 69], F32, kind="ExternalInput")
    v_in = nc.dram_tensor("vv", [BPC, NVT, SUB, VT_COLS * DV], BF16, kind="ExternalInput")
    out_d = nc.dram_tensor("out", [128, BPC * DV], F32, kind="ExternalOutput")

    with TileContext(nc) as tc:
        with (
            tc.tile_pool(name="const", bufs=1) as cpool,
            tc.tile_pool(name="kp", bufs=6) as kpool,
            tc.tile_pool(name="vp_", bufs=2 * NVT) as vpool,
            tc.tile_pool(name="hp", bufs=3) as hpool,
            tc.tile_pool(name="wp", bufs=2) as wpool,
            tc.tile_pool(name="ob", bufs=1) as opool,
            tc.tile_pool(name="pre", bufs=3, space="PSUM") as pre_pool,
            tc.tile_pool(name="sps", bufs=2, space="PSUM") as s_pool,
            tc.tile_pool(name="ops", bufs=2, space="PSUM") as o_pool,
        ):
            # HAM warm-up on zeroed tiles: needs no DMA, so the PE clock
            # gate lifts during the Tile preamble / first transfers.
            zwarm = cpool.tile([128, 512], BF16)
            nc.gpsimd.memset(zwarm[:], 0.0)
            warm_ps = pre_pool.tile([H, SC], F32, tag="pre")
            for _ in range(16):
                nc.tensor.matmul(
                    warm_ps[:, :512], zwarm[:, :128], zwarm[:], start=True, stop=True
                )

            cst = cpool.tile([128, 69], F32)
            nc.sync.dma_start(cst[:], cst_in[:])
            qwq = cst[:, 0:4]
            wk = cst[:, 4:68].bitcast(BF16)
            vph = cst[:, 68:69].bitcast(BF16)[:, 0:1]

            out_sb = opool.tile([128, BPC * DV], F32)

            def load_ktile(b, g):
                ktile = kpool.tile([D, KTILE], BF16, tag="kt")
                nc.sync.dma_start(ktile[:], kh_in[b, :, g * KTILE : (g + 1) * KTILE])
                return ktile

            for b in range(BPC):
                # v~ tiles for this batch (SWDGE queue so the large v
                # prefetches never head-of-line-block the kT stream, whose
                # issue rate is throttled by PE back-pressure)
                kts = {}
                if b == 0:
                    kts[0] = load_ktile(0, 0)
                    v_tiles = [None] * NVT
                else:
                    v_tiles = next_v_tiles
                next_v_tiles = [None] * NVT

                acc = o_pool.tile([128, DV], F32, tag="acc")
                nc.vector.memset(acc[:], 0.0)
                w = wpool.tile([SUB, L // SUB], BF16, tag="w")
                for t in range(NSC):
                    if t % KSC == 0 and t // KSC not in kts:
                        kts[t // KSC] = load_ktile(b, t // KSC)
                    # batch 0 pulls its own v spread through its front half
                    if b == 0 and t % 2 == 0 and t // 2 < NVT:
                        vtile = vpool.tile([SUB, VT_COLS * DV], BF16, tag="vt")
                        nc.gpsimd.dma_start(vtile[:], v_in[0, t // 2])
                        v_tiles[t // 2] = vtile
                    # prefetch next batch's v in the back half of this batch
                    if t >= NSC - 2 * NVT and (t - (NSC - 2 * NVT)) % 2 == 0 and b + 1 < BPC:
                        vt = (t - (NSC - 2 * NVT)) // 2
                        vtile = vpool.tile([SUB, VT_COLS * DV], BF16, tag="vt")
                        nc.gpsimd.dma_start(vtile[:], v_in[b + 1, vt])
                        next_v_tiles[vt] = vtile

                    pre = pre_pool.tile([H, SC], F32, tag="pre")
                    kt = kts[t // KSC]
                    off = (t % KSC) * SC
                    nc.tensor.matmul(
                        pre[:], wk[:], kt[:, off : off + SC], start=True, stop=True
                    )

                    hh = hpool.tile([H, SC], BF16, tag="hh")
                    nc.scalar.activation(
                        hh[:], pre[:], ACTF.Tanh, bias=qwq[:, b : b + 1], scale=1.0
                    )
                    if t % 4 == 0:
                        scol4 = s_pool.tile([SUB, 4 * NSUB], F32, tag="scol")
                    scol = scol4[:, (t % 4) * NSUB : (t % 4 + 1) * NSUB]
                    for j in range(NSUB):
                        js = slice(j * SUB, (j + 1) * SUB)
                        nc.tensor.matmul(
                            scol[:, j : j + 1], hh[:, js], vph[:],
                            start=True, stop=True,
                        )
                    if t % 4 == 3:
                        g = t // 4  # w col group 16g..16g+15 == v tile g
                        nc.scalar.activation(
                            w[:, 16 * g : 16 * (g + 1)], scol4[:], ACTF.Exp
                        )
                        # W3 for this group, 4 col-group-packed chains
                        for col in range(VT_COLS):
                            tp = 16 * g + col
                            c = tp % NCHAIN
                            nc.tensor.matmul(
                                acc[32 * c : 32 * c + 1, :],
                                w[:, tp : tp + 1],
                                v_tiles[g][:, col * DV : (col + 1) * DV],
                                start=(tp < NCHAIN),
                                stop=(tp >= L // SUB - NCHAIN),
                                tile_position=(0, 32 * c),
                            )
                nc.vector.tensor_copy(out_sb[:, b * DV : (b + 1) * DV], acc[:])

            nc.sync.dma_start(out_d[:], out_sb[:])

    _split_excess_waits(nc)
    return nc


def _prep_inputs(q, k, v, W_line, v_param):
    """Host-side shard + layout prep. Returns per-core input maps."""
    bf = ml_dtypes.bfloat16
    qWq = q.astype(np.float64) @ W_line[:D].astype(np.float64)  # (B, H)
    wk = np.ascontiguousarray(W_line[D:]).astype(bf)  # (D, H) bf16
    vp_pad = np.zeros((H, 2), dtype=bf)
    vp_pad[:, 0] = v_param.astype(bf)

    cst_base = np.zeros((128, 69), dtype=np.float32)
    cst_base[:, 4:68] = wk.view(np.float32)
    cst_base[:, 68:69] = vp_pad.view(np.float32)

    in_maps = []
    for c in range(NCORES):
        bs = slice(c * BPC, (c + 1) * BPC)
        kh = np.ascontiguousarray(k[bs].transpose(0, 2, 1).astype(bf))  # (BPC, D, L)
        vv = np.zeros((BPC, L, DV), dtype=np.float32)
        vv[:, :, :D] = v[bs]
        vv[:, :, D] = 1.0
        # permute into the SBUF tile layout: [b][vt][p][t*DV+d]
        vv = np.ascontiguousarray(
            vv.reshape(BPC, NVT, VT_COLS, SUB, DV)
            .transpose(0, 1, 3, 2, 4)
            .reshape(BPC, NVT, SUB, VT_COLS * DV)
            .astype(bf)
        )
        cst = cst_base.copy()
        cst[:, 0:4] = qWq[bs].T.astype(np.float32)  # (H, BPC)
        in_maps.append({"kh": kh, "vv": vv, "cst": cst})
    return in_maps


def _gather_output(results):
    out = np.empty((B, D), dtype=np.float32)
    for c, r in enumerate(results):
        rows = r["out"].reshape(128, BPC, DV).astype(np.float64)
        # sum the 4 col-group partial accumulators (partitions 0/32/64/96)
        s = rows[0] + rows[32] + rows[64] + rows[96]  # (BPC, DV)
        out[c * BPC : (c + 1) * BPC] = (s[:, :D] / s[:, D : D + 1]).astype(np.float32)
    return out


def run(q, k, v, W_line, v_param, trace=False, **spmd_kwargs):
    from concourse.bass_utils import run_bass_kernel_spmd

    if "nc" not in _CACHE:
        _CACHE["nc"] = build_nc()
    nc = _CACHE["nc"]
    in_maps = _prep_inputs(q, k, v, W_line, v_param)
    res = run_bass_kernel_spmd(
        nc, in_maps, list(range(NCORES)), trace=trace, **spmd_kwargs
    )
    return _gather_output(res.results), res


def kernel(q, k, v, W_line, v_param):
    out, _ = run(q, k, v, W_line, v_param, trace=False)
    return out
